# revision 6
# baseline (speedup 1.0000x reference)
"""Trainium2 kernel for nn_ContrastiveLoss_matrix (cross-attention i2t contrastive loss).

Strategy (8 NeuronCores, SPMD, caption-axis sharding):
  - Each core computes scores[:, m*16:(m+1)*16] (16 captions) for both the
    (im, s) and (pred, s_pred) terms.
  - Einsum 'brd,cwd->bcrw' is a [BR, D] @ [D, CW] matmul done in f32r
    (full-rate fp32 variant) with D on the contraction (partition) axis.
  - Word masking is pre-applied on the host by replacing padded caption
    words with a copy of word 0 (max over words is then exactly the max
    over valid words).
  - max over words: DVE segmented tensor_reduce over the PSUM tile viewed
    as [128, ncap, W].
  - sum over regions: PE matmul with a 0/1 indicator matrix A[br, b]
    (A.T @ mx accumulated over all row tiles), software-pipelined one
    iteration behind the main matmuls so the PE never waits on the DVE.
  - part 2 (pred) runs first: its caption shard is the smallest transfer,
    so compute starts earliest while the bigger part-1 operands stream in.
  - host applies the 1/(obj_num+eps) scaling, gathers the 8 shards into
    scores [128, 128], and computes the (tiny) margin loss in numpy.
"""

import numpy as np

B = 128
R = 36
W = 50
Rp = 25
Wp = 30
D = 1024
P = 128
KO = D // P          # 8 contraction chunks
NCORES = 8
CSH = B // NCORES    # 16 captions per core
EPS = 1e-6
MARGIN = 0.2

BR = B * R           # 4608  -> 36 row tiles
BRp = B * Rp         # 3200  -> 25 row tiles
NT1 = BR // P        # 36
NT2 = BRp // P       # 25
COLS1 = CSH * W      # 800
COLS2 = CSH * Wp     # 480
HALF1 = COLS1 // 2   # 400 (8 captions * 50 words per half)

_NC = None


def _chunks(n_tiles, first, step):
    """[(t0, ntiles), ...] covering range(n_tiles); first chunk small."""
    out = []
    t = 0
    size = first
    while t < n_tiles:
        sz = min(size, n_tiles - t)
        out.append((t, sz))
        t += sz
        size = step
    return out


def _build():
    import concourse.tile as tile
    from concourse import bacc, mybir

    f32 = mybir.dt.float32
    f32r = mybir.dt.float32r
    AX = mybir.AxisListType
    OP = mybir.AluOpType

    nc = bacc.Bacc("TRN2", target_bir_lowering=False, debug=False)

    imT = nc.dram_tensor("imT", [D, BR], f32r, kind="ExternalInput").ap()
    predT = nc.dram_tensor("predT", [D, BRp], f32r, kind="ExternalInput").ap()
    sT = nc.dram_tensor("sT", [D, COLS1], f32r, kind="ExternalInput").ap()
    spT = nc.dram_tensor("spT", [D, COLS2], f32r, kind="ExternalInput").ap()
    Aim = nc.dram_tensor("Aim", [BR, P], f32, kind="ExternalInput").ap()
    Apred = nc.dram_tensor("Apred", [BRp, P], f32, kind="ExternalInput").ap()
    out = nc.dram_tensor("sims", [P, 2 * CSH], f32, kind="ExternalOutput").ap()

    imT_r = imT.rearrange("(ko p) n -> p ko n", p=P)      # [128, 8, 4608]
    predT_r = predT.rearrange("(ko p) n -> p ko n", p=P)  # [128, 8, 3200]
    sT_r = sT.rearrange("(ko p) n -> p ko n", p=P)        # [128, 8, 800]
    spT_r = spT.rearrange("(ko p) n -> p ko n", p=P)      # [128, 8, 480]
    Aim_r = Aim.rearrange("(t p) m -> p t m", p=P)        # [128, 36, 128]
    Apred_r = Apred.rearrange("(t p) m -> p t m", p=P)    # [128, 25, 128]

    IM_CHUNKS = _chunks(NT1, 1, 4)    # first chunk 1 tile for fast start
    PRED_CHUNKS = _chunks(NT2, 1, 4)

    with tile.TileContext(nc) as tc:
        with (
            tc.tile_pool(name="const", bufs=1) as const_pool,
            tc.tile_pool(name="imt", bufs=3) as im_pool,
            tc.tile_pool(name="mx", bufs=4) as mx_pool,
            tc.tile_pool(name="fin", bufs=1) as fin_pool,
            tc.tile_pool(name="mm", bufs=6, space="PSUM") as psum_mm,
            tc.tile_pool(name="sim", bufs=1, space="PSUM") as psum_sim,
        ):
            # resident operands — DMA queue order is program order, so emit
            # transfers in the order compute consumes them: part-2 operands
            # first, then part-1's big shards stream in behind them.
            sp_sb = const_pool.tile([P, KO, COLS2], f32r)
            for ko in range(0, KO, 2):   # small chunks -> compute starts early
                nc.sync.dma_start(sp_sb[:, ko:ko + 2, :], spT_r[:, ko:ko + 2, :])
            s_sb = const_pool.tile([P, KO, COLS1], f32r)
            a_all = const_pool.tile([P, NT1, P], f32)

            sim1 = psum_sim.tile([P, CSH], f32, tag="sim1")
            sim2 = psum_sim.tile([P, CSH], f32, tag="sim2")

            # ---- part 2 first: pred x s_pred (smallest startup transfer) ----
            pend = None  # (t, mx) of previous row tile
            chunk_tiles = {}
            for ci, (t0, ncht) in enumerate(PRED_CHUNKS):
                ch = im_pool.tile([P, KO, 4 * P], f32r, tag="imtile")
                nc.sync.dma_start(ch[:, :, :ncht * P],
                                  predT_r[:, :, t0 * P:(t0 + ncht) * P])
                for i in range(ncht):
                    chunk_tiles[t0 + i] = (ch, i)
                if ci == 0:
                    ap_all = const_pool.tile([P, NT2, P], f32)
                    nc.sync.dma_start(ap_all[:], Apred_r)
            # part-1 operands queue up behind everything part 2 needs
            for ko in range(0, KO, 2):
                nc.sync.dma_start(s_sb[:, ko:ko + 2, :], sT_r[:, ko:ko + 2, :])
            nc.sync.dma_start(a_all[:], Aim_r)
            for t in range(NT2):
                ch, i = chunk_tiles[t]
                mx = mx_pool.tile([P, CSH], f32, tag="mx")
                ps = psum_mm.tile([P, COLS2], f32, tag="mm")
                for ko in range(KO):
                    nc.tensor.matmul(
                        ps[:],
                        ch[:, ko, i * P:(i + 1) * P],
                        sp_sb[:, ko, :],
                        start=(ko == 0),
                        stop=(ko == KO - 1),
                    )
                nc.vector.tensor_reduce(
                    out=mx[:],
                    in_=ps.rearrange("p (c w) -> p c w", w=Wp),
                    axis=AX.X,
                    op=OP.max,
                )
                if pend is not None:
                    nc.tensor.matmul(sim2[:], ap_all[:, pend[0], :], pend[1][:],
                                     start=(t == 1), stop=False)
                pend = (t, mx)
            nc.tensor.matmul(sim2[:], ap_all[:, pend[0], :], pend[1][:],
                             start=(NT2 == 1), stop=True)

            # ---- part 1: im x s ----
            pend = None
            chunk_tiles = {}
            for t0, ncht in IM_CHUNKS:
                ch = im_pool.tile([P, KO, 4 * P], f32r, tag="imtile")
                nc.sync.dma_start(ch[:, :, :ncht * P],
                                  imT_r[:, :, t0 * P:(t0 + ncht) * P])
                for i in range(ncht):
                    chunk_tiles[t0 + i] = (ch, i)
            for t in range(NT1):
                ch, i = chunk_tiles[t]
                mx = mx_pool.tile([P, CSH], f32, tag="mx")
                for h in range(2):
                    ps = psum_mm.tile([P, HALF1], f32, tag="mm")
                    for ko in range(KO):
                        nc.tensor.matmul(
                            ps[:],
                            ch[:, ko, i * P:(i + 1) * P],
                            s_sb[:, ko, h * HALF1:(h + 1) * HALF1],
                            start=(ko == 0),
                            stop=(ko == KO - 1),
                        )
                    nc.vector.tensor_reduce(
                        out=mx[:, h * 8:(h + 1) * 8],
                        in_=ps.rearrange("p (c w) -> p c w", w=W),
                        axis=AX.X,
                        op=OP.max,
                    )
                if pend is not None:
                    nc.tensor.matmul(sim1[:], a_all[:, pend[0], :], pend[1][:],
                                     start=(t == 1), stop=False)
                pend = (t, mx)
            nc.tensor.matmul(sim1[:], a_all[:, pend[0], :], pend[1][:],
                             start=(NT1 == 1), stop=True)

            # ---- tail: copy the two accumulators out (scaling on host) ----
            o = fin_pool.tile([P, 2 * CSH], f32)
            nc.scalar.copy(o[:, :CSH], sim1[:])
            nc.scalar.copy(o[:, CSH:], sim2[:])
            nc.sync.dma_start(out, o[:])

    nc.compile()
    return nc


def _get_nc():
    global _NC
    if _NC is None:
        _NC = _build()
    return _NC


def _dup_pad_words(x, lens, width):
    # replace padded words with a copy of word 0 so that max over all words
    # == max over valid words (every row has >= 1 valid word)
    pad = np.arange(width)[None, :] >= lens[:, None]          # [B, W]
    return np.where(pad[:, :, None], x[:, :1, :], x)


LAST_RESULT = None


def kernel(im, im_l, s, s_l, pred, pred_l, s_pred, s_pred_l, _trace=False):
    from concourse.bass_utils import run_bass_kernel_spmd

    global LAST_RESULT
    im = np.asarray(im, np.float32)
    s = np.asarray(s, np.float32)
    pred = np.asarray(pred, np.float32)
    s_pred = np.asarray(s_pred, np.float32)
    im_l = np.asarray(im_l)
    s_l = np.asarray(s_l)
    pred_l = np.asarray(pred_l)
    s_pred_l = np.asarray(s_pred_l)

    s_fix = _dup_pad_words(s, s_l, W)
    sp_fix = _dup_pad_words(s_pred, s_pred_l, Wp)

    imT = np.ascontiguousarray(im.reshape(BR, D).T)
    predT = np.ascontiguousarray(pred.reshape(BRp, D).T)
    sT = np.ascontiguousarray(s_fix.reshape(B * W, D).T)      # [D, 6400]
    spT = np.ascontiguousarray(sp_fix.reshape(B * Wp, D).T)   # [D, 3840]

    Aim = np.zeros((BR, P), np.float32)
    Aim[np.arange(BR), np.arange(BR) // R] = 1.0
    Apred = np.zeros((BRp, P), np.float32)
    Apred[np.arange(BRp), np.arange(BRp) // Rp] = 1.0

    inv_im = (np.float32(1.0) / (im_l.astype(np.float32) + np.float32(EPS)))
    inv_pred = (np.float32(1.0) / (pred_l.astype(np.float32) + np.float32(EPS)))

    in_maps = []
    for m in range(NCORES):
        in_maps.append({
            "imT": imT,
            "predT": predT,
            "sT": np.ascontiguousarray(sT[:, m * COLS1:(m + 1) * COLS1]),
            "spT": np.ascontiguousarray(spT[:, m * COLS2:(m + 1) * COLS2]),
            "Aim": Aim,
            "Apred": Apred,
        })

    nc = _get_nc()
    res = run_bass_kernel_spmd(
        nc, in_maps, core_ids=list(range(NCORES)), trace=_trace,
        trace_cores=list(range(NCORES)) if _trace else None,
    )
    LAST_RESULT = res

    shards = []
    for m in range(NCORES):
        sims = res.results[m]["sims"]
        shard = sims[:, :CSH] * inv_im[:, None] + sims[:, CSH:] * inv_pred[:, None]
        shards.append(shard.astype(np.float32))
    scores = np.concatenate(shards, axis=1)

    diag = np.diagonal(scores).copy()[:, None]                 # [B, 1]
    cost_s = np.clip(MARGIN + scores - diag, 0.0, None)
    cost_im = np.clip(MARGIN + scores - diag.T, 0.0, None)
    np.fill_diagonal(cost_s, 0.0)
    np.fill_diagonal(cost_im, 0.0)
    loss = np.float32(cost_s.sum(dtype=np.float32) + cost_im.sum(dtype=np.float32))
    return loss, scores


# revision 8
# speedup vs baseline: 1.0119x; 1.0119x over previous
"""Trainium2 kernel for nn_ContrastiveLoss_matrix (cross-attention i2t contrastive loss).

Strategy (8 NeuronCores, SPMD, caption-axis sharding):
  - Each core computes scores[:, m*16:(m+1)*16] (16 captions) for both the
    (im, s) and (pred, s_pred) terms.
  - Einsum 'brd,cwd->bcrw' is a [BR, D] @ [D, CW] matmul done in f32r
    (full-rate fp32 variant) with D on the contraction (partition) axis.
  - Word masking is pre-applied on the host by replacing padded caption
    words with a copy of word 0 (max over words is then exactly the max
    over valid words).
  - max over words: DVE segmented tensor_reduce over the PSUM tile viewed
    as [128, ncap, W].
  - sum over regions: PE matmul with a 0/1 indicator matrix A[br, b]
    (A.T @ mx accumulated over all row tiles), software-pipelined one
    iteration behind the main matmuls so the PE never waits on the DVE.
  - part 2 (pred) runs first: its caption shard is the smallest transfer,
    so compute starts earliest while the bigger part-1 operands stream in.
  - host applies the 1/(obj_num+eps) scaling, gathers the 8 shards into
    scores [128, 128], and computes the (tiny) margin loss in numpy.
"""

import numpy as np

B = 128
R = 36
W = 50
Rp = 25
Wp = 30
D = 1024
P = 128
KO = D // P          # 8 contraction chunks
NCORES = 8
CSH = B // NCORES    # 16 captions per core
EPS = 1e-6
MARGIN = 0.2

BR = B * R           # 4608  -> 36 row tiles
BRp = B * Rp         # 3200  -> 25 row tiles
NT1 = BR // P        # 36
NT2 = BRp // P       # 25
COLS1 = CSH * W      # 800
COLS2 = CSH * Wp     # 480
HALF1 = COLS1 // 2   # 400 (8 captions * 50 words per half)

_NC = None


def _chunks(n_tiles, first, step):
    """[(t0, ntiles), ...] covering range(n_tiles); first chunk small."""
    out = []
    t = 0
    size = first
    while t < n_tiles:
        sz = min(size, n_tiles - t)
        out.append((t, sz))
        t += sz
        size = step
    return out


def _build():
    import concourse.tile as tile
    from concourse import bacc, mybir

    f32 = mybir.dt.float32
    f32r = mybir.dt.float32r
    AX = mybir.AxisListType
    OP = mybir.AluOpType

    nc = bacc.Bacc("TRN2", target_bir_lowering=False, debug=False)

    imT = nc.dram_tensor("imT", [D, BR], f32r, kind="ExternalInput").ap()
    predT = nc.dram_tensor("predT", [D, BRp], f32r, kind="ExternalInput").ap()
    sT = nc.dram_tensor("sT", [D, COLS1], f32r, kind="ExternalInput").ap()
    spT = nc.dram_tensor("spT", [D, COLS2], f32r, kind="ExternalInput").ap()
    Aim = nc.dram_tensor("Aim", [BR, P], f32, kind="ExternalInput").ap()
    Apred = nc.dram_tensor("Apred", [BRp, P], f32, kind="ExternalInput").ap()
    out = nc.dram_tensor("sims", [P, 2 * CSH], f32, kind="ExternalOutput").ap()

    imT_r = imT.rearrange("(ko p) n -> p ko n", p=P)      # [128, 8, 4608]
    predT_r = predT.rearrange("(ko p) n -> p ko n", p=P)  # [128, 8, 3200]
    sT_r = sT.rearrange("(ko p) n -> p ko n", p=P)        # [128, 8, 800]
    spT_r = spT.rearrange("(ko p) n -> p ko n", p=P)      # [128, 8, 480]
    Aim_r = Aim.rearrange("(t p) m -> p t m", p=P)        # [128, 36, 128]
    Apred_r = Apred.rearrange("(t p) m -> p t m", p=P)    # [128, 25, 128]

    IM_CHUNKS = _chunks(NT1, 1, 4)    # first chunk 1 tile for fast start
    PRED_CHUNKS = _chunks(NT2, 1, 4)

    with tile.TileContext(nc) as tc:
        with (
            tc.tile_pool(name="const", bufs=1) as const_pool,
            tc.tile_pool(name="imt", bufs=6) as im_pool,
            tc.tile_pool(name="mx", bufs=4) as mx_pool,
            tc.tile_pool(name="fin", bufs=1) as fin_pool,
            tc.tile_pool(name="mm", bufs=6, space="PSUM") as psum_mm,
            tc.tile_pool(name="sim", bufs=1, space="PSUM") as psum_sim,
        ):
            # resident operands — DMA queue order is program order, so emit
            # transfers in the order compute consumes them: part-2 operands
            # first, then part-1's big shards stream in behind them.
            sp_sb = const_pool.tile([P, KO, COLS2], f32r)
            for ko in range(0, KO, 2):   # small chunks -> compute starts early
                nc.sync.dma_start(sp_sb[:, ko:ko + 2, :], spT_r[:, ko:ko + 2, :])
            s_sb = const_pool.tile([P, KO, COLS1], f32r)
            a_all = const_pool.tile([P, NT1, P], f32)

            sim1 = psum_sim.tile([P, CSH], f32, tag="sim1")
            sim2 = psum_sim.tile([P, CSH], f32, tag="sim2")

            # ---- part 2 first: pred x s_pred (smallest startup transfer) ----
            # DMA prologue: interleave part-1's s-shard chunks between the
            # pred chunks so both arrive just ahead of their consumers
            # without head-of-line blocking the (serialized) DMA path.
            pend = None  # (t, mx) of previous row tile
            chunk_tiles = {}
            s_ko = 0

            def emit_pred_chunk(t0, ncht):
                ch = im_pool.tile([P, KO, 4 * P], f32r, tag="imtile")
                nc.sync.dma_start(ch[:, :, :ncht * P],
                                  predT_r[:, :, t0 * P:(t0 + ncht) * P])
                for i in range(ncht):
                    chunk_tiles[t0 + i] = (ch, i)

            def emit_s_chunk():
                nonlocal s_ko
                if s_ko < KO:
                    nc.sync.dma_start(s_sb[:, s_ko:s_ko + 2, :],
                                      sT_r[:, s_ko:s_ko + 2, :])
                    s_ko += 2

            emit_pred_chunk(*PRED_CHUNKS[0])
            ap_all = const_pool.tile([P, NT2, P], f32)
            nc.sync.dma_start(ap_all[:], Apred_r)
            emit_pred_chunk(*PRED_CHUNKS[1])
            emit_pred_chunk(*PRED_CHUNKS[2])
            for ci in range(3, len(PRED_CHUNKS) - 1):
                emit_s_chunk()
                emit_pred_chunk(*PRED_CHUNKS[ci])
            emit_s_chunk()
            emit_s_chunk()
            emit_s_chunk()
            nc.sync.dma_start(a_all[:], Aim_r)
            emit_pred_chunk(*PRED_CHUNKS[-1])
            for t in range(NT2):
                ch, i = chunk_tiles[t]
                mx = mx_pool.tile([P, CSH], f32, tag="mx")
                ps = psum_mm.tile([P, COLS2], f32, tag="mm")
                for ko in range(KO):
                    nc.tensor.matmul(
                        ps[:],
                        ch[:, ko, i * P:(i + 1) * P],
                        sp_sb[:, ko, :],
                        start=(ko == 0),
                        stop=(ko == KO - 1),
                    )
                nc.vector.tensor_reduce(
                    out=mx[:],
                    in_=ps.rearrange("p (c w) -> p c w", w=Wp),
                    axis=AX.X,
                    op=OP.max,
                )
                if pend is not None:
                    nc.tensor.matmul(sim2[:], ap_all[:, pend[0], :], pend[1][:],
                                     start=(t == 1), stop=False)
                pend = (t, mx)
            nc.tensor.matmul(sim2[:], ap_all[:, pend[0], :], pend[1][:],
                             start=(NT2 == 1), stop=True)

            # ---- part 1: im x s ----
            pend = None
            chunk_tiles = {}
            for t0, ncht in IM_CHUNKS:
                ch = im_pool.tile([P, KO, 4 * P], f32r, tag="imtile")
                nc.sync.dma_start(ch[:, :, :ncht * P],
                                  imT_r[:, :, t0 * P:(t0 + ncht) * P])
                for i in range(ncht):
                    chunk_tiles[t0 + i] = (ch, i)
            for t in range(NT1):
                ch, i = chunk_tiles[t]
                mx = mx_pool.tile([P, CSH], f32, tag="mx")
                for h in range(2):
                    ps = psum_mm.tile([P, HALF1], f32, tag="mm")
                    for ko in range(KO):
                        nc.tensor.matmul(
                            ps[:],
                            ch[:, ko, i * P:(i + 1) * P],
                            s_sb[:, ko, h * HALF1:(h + 1) * HALF1],
                            start=(ko == 0),
                            stop=(ko == KO - 1),
                        )
                    nc.vector.tensor_reduce(
                        out=mx[:, h * 8:(h + 1) * 8],
                        in_=ps.rearrange("p (c w) -> p c w", w=W),
                        axis=AX.X,
                        op=OP.max,
                    )
                if pend is not None:
                    nc.tensor.matmul(sim1[:], a_all[:, pend[0], :], pend[1][:],
                                     start=(t == 1), stop=False)
                pend = (t, mx)
            nc.tensor.matmul(sim1[:], a_all[:, pend[0], :], pend[1][:],
                             start=(NT1 == 1), stop=True)

            # ---- tail: copy the two accumulators out (scaling on host) ----
            o = fin_pool.tile([P, 2 * CSH], f32)
            nc.scalar.copy(o[:, :CSH], sim1[:])
            nc.scalar.copy(o[:, CSH:], sim2[:])
            nc.sync.dma_start(out, o[:])

    nc.compile()
    return nc


def _get_nc():
    global _NC
    if _NC is None:
        _NC = _build()
    return _NC


def _dup_pad_words(x, lens, width):
    # replace padded words with a copy of word 0 so that max over all words
    # == max over valid words (every row has >= 1 valid word)
    pad = np.arange(width)[None, :] >= lens[:, None]          # [B, W]
    return np.where(pad[:, :, None], x[:, :1, :], x)


LAST_RESULT = None


def kernel(im, im_l, s, s_l, pred, pred_l, s_pred, s_pred_l, _trace=False):
    from concourse.bass_utils import run_bass_kernel_spmd

    global LAST_RESULT
    im = np.asarray(im, np.float32)
    s = np.asarray(s, np.float32)
    pred = np.asarray(pred, np.float32)
    s_pred = np.asarray(s_pred, np.float32)
    im_l = np.asarray(im_l)
    s_l = np.asarray(s_l)
    pred_l = np.asarray(pred_l)
    s_pred_l = np.asarray(s_pred_l)

    s_fix = _dup_pad_words(s, s_l, W)
    sp_fix = _dup_pad_words(s_pred, s_pred_l, Wp)

    imT = np.ascontiguousarray(im.reshape(BR, D).T)
    predT = np.ascontiguousarray(pred.reshape(BRp, D).T)
    sT = np.ascontiguousarray(s_fix.reshape(B * W, D).T)      # [D, 6400]
    spT = np.ascontiguousarray(sp_fix.reshape(B * Wp, D).T)   # [D, 3840]

    Aim = np.zeros((BR, P), np.float32)
    Aim[np.arange(BR), np.arange(BR) // R] = 1.0
    Apred = np.zeros((BRp, P), np.float32)
    Apred[np.arange(BRp), np.arange(BRp) // Rp] = 1.0

    inv_im = (np.float32(1.0) / (im_l.astype(np.float32) + np.float32(EPS)))
    inv_pred = (np.float32(1.0) / (pred_l.astype(np.float32) + np.float32(EPS)))

    in_maps = []
    for m in range(NCORES):
        in_maps.append({
            "imT": imT,
            "predT": predT,
            "sT": np.ascontiguousarray(sT[:, m * COLS1:(m + 1) * COLS1]),
            "spT": np.ascontiguousarray(spT[:, m * COLS2:(m + 1) * COLS2]),
            "Aim": Aim,
            "Apred": Apred,
        })

    nc = _get_nc()
    res = run_bass_kernel_spmd(
        nc, in_maps, core_ids=list(range(NCORES)), trace=_trace,
        trace_cores=list(range(NCORES)) if _trace else None,
    )
    LAST_RESULT = res

    shards = []
    for m in range(NCORES):
        sims = res.results[m]["sims"]
        shard = sims[:, :CSH] * inv_im[:, None] + sims[:, CSH:] * inv_pred[:, None]
        shards.append(shard.astype(np.float32))
    scores = np.concatenate(shards, axis=1)

    diag = np.diagonal(scores).copy()[:, None]                 # [B, 1]
    cost_s = np.clip(MARGIN + scores - diag, 0.0, None)
    cost_im = np.clip(MARGIN + scores - diag.T, 0.0, None)
    np.fill_diagonal(cost_s, 0.0)
    np.fill_diagonal(cost_im, 0.0)
    loss = np.float32(cost_s.sum(dtype=np.float32) + cost_im.sum(dtype=np.float32))
    return loss, scores


# revision 9
# speedup vs baseline: 1.0777x; 1.0649x over previous
"""Trainium2 kernel for nn_ContrastiveLoss_matrix (cross-attention i2t contrastive loss).

Strategy (8 NeuronCores, SPMD, caption-axis sharding + ragged packing):
  - Captions are sorted by length (desc) and dealt round-robin into 16
    slots x 8 cores; slots are grouped into 4 equal-width buckets whose
    widths come from the actual length tensor at kernel-call time (the
    NEFF is compiled per width signature). This packs away ~40% of the
    padded-word matmul columns while every core runs the same kernel.
  - Einsum 'brd,cwd->bcrw' is a [BR, D] @ [D, cols] matmul in f32r
    (full-rate fp32 variant), D on the contraction (partition) axis.
  - Word masking: padded words are replaced host-side with a copy of
    word 0, so max over all packed words == max over valid words.
  - max over words: DVE segmented tensor_reduce per equal-width slot run.
  - sum over regions: PE matmul with a 0/1 indicator A[br, b] accumulated
    over row tiles, software-pipelined one iteration behind the main
    matmuls so the PE never waits on the DVE.
  - part 2 (pred) runs first (smallest startup transfer); part-1 operands
    stream in behind it with hand-interleaved DMA ordering.
  - host applies 1/(obj_num+eps), unpermutes the sorted caption columns,
    and computes the (tiny) margin loss in numpy.
"""

import numpy as np

B = 128
R = 36
W = 50
Rp = 25
Wp = 30
D = 1024
P = 128
KO = D // P          # 8 contraction chunks
NCORES = 8
CSH = B // NCORES    # 16 slots (captions) per core
NBUCK = 4            # slots per bucket = CSH // NBUCK
EPS = 1e-6
MARGIN = 0.2

BR = B * R           # 4608  -> 36 row tiles
BRp = B * Rp         # 3200  -> 25 row tiles
NT1 = BR // P        # 36
NT2 = BRp // P       # 25

_NC_CACHE = {}


def _chunks(n_tiles, first, step):
    """[(t0, ntiles), ...] covering range(n_tiles); first chunk small."""
    out = []
    t = 0
    size = first
    while t < n_tiles:
        sz = min(size, n_tiles - t)
        out.append((t, sz))
        t += sz
        size = step
    return out


def _slot_widths(bucket_w):
    return [bucket_w[s // (CSH // NBUCK)] for s in range(CSH)]


def _plan(widths):
    """Greedy-pack the 16 slots into PSUM tiles of <= 512 columns.

    Returns (tiles, total_cols); each tile is
    {'col0', 'cols', 'segs': [(slot0, nslots, width, rel_off), ...]}
    with segs being runs of equal-width slots."""
    tiles = []
    col = 0
    cur = None
    for s, wv in enumerate(widths):
        if cur is None or cur["cols"] + wv > 512:
            cur = {"col0": col, "cols": 0, "segs": []}
            tiles.append(cur)
        if cur["segs"] and cur["segs"][-1][2] == wv:
            s0, n, _, off = cur["segs"][-1]
            cur["segs"][-1] = (s0, n + 1, wv, off)
        else:
            cur["segs"].append((s, 1, wv, cur["cols"]))
        cur["cols"] += wv
        col += wv
    return tiles, col


def _build(w1, w2):
    import concourse.tile as tile
    from concourse import bacc, mybir

    f32 = mybir.dt.float32
    f32r = mybir.dt.float32r
    AX = mybir.AxisListType
    OP = mybir.AluOpType

    TILES1, COLS1 = _plan(_slot_widths(w1))
    TILES2, COLS2 = _plan(_slot_widths(w2))

    nc = bacc.Bacc("TRN2", target_bir_lowering=False, debug=False)

    imT = nc.dram_tensor("imT", [D, BR], f32r, kind="ExternalInput").ap()
    predT = nc.dram_tensor("predT", [D, BRp], f32r, kind="ExternalInput").ap()
    sT = nc.dram_tensor("sT", [D, COLS1], f32r, kind="ExternalInput").ap()
    spT = nc.dram_tensor("spT", [D, COLS2], f32r, kind="ExternalInput").ap()
    Aim = nc.dram_tensor("Aim", [BR, P], f32, kind="ExternalInput").ap()
    Apred = nc.dram_tensor("Apred", [BRp, P], f32, kind="ExternalInput").ap()
    out = nc.dram_tensor("sims", [P, 2 * CSH], f32, kind="ExternalOutput").ap()

    imT_r = imT.rearrange("(ko p) n -> p ko n", p=P)      # [128, 8, 4608]
    predT_r = predT.rearrange("(ko p) n -> p ko n", p=P)  # [128, 8, 3200]
    sT_r = sT.rearrange("(ko p) n -> p ko n", p=P)        # [128, 8, C1]
    spT_r = spT.rearrange("(ko p) n -> p ko n", p=P)      # [128, 8, C2]
    Aim_r = Aim.rearrange("(t p) m -> p t m", p=P)        # [128, 36, 128]
    Apred_r = Apred.rearrange("(t p) m -> p t m", p=P)    # [128, 25, 128]

    IM_CHUNKS = _chunks(NT1, 1, 4)    # first chunk 1 tile for fast start
    PRED_CHUNKS = _chunks(NT2, 1, 4)

    with tile.TileContext(nc) as tc:
        with (
            tc.tile_pool(name="const", bufs=1) as const_pool,
            tc.tile_pool(name="imt", bufs=6) as im_pool,
            tc.tile_pool(name="mx", bufs=4) as mx_pool,
            tc.tile_pool(name="fin", bufs=1) as fin_pool,
            tc.tile_pool(name="mm", bufs=6, space="PSUM") as psum_mm,
            tc.tile_pool(name="sim", bufs=1, space="PSUM") as psum_sim,
        ):
            sp_sb = const_pool.tile([P, KO, COLS2], f32r)
            for ko in range(0, KO, 2):   # small chunks -> compute starts early
                nc.sync.dma_start(sp_sb[:, ko:ko + 2, :], spT_r[:, ko:ko + 2, :])
            s_sb = const_pool.tile([P, KO, COLS1], f32r)
            a_all = const_pool.tile([P, NT1, P], f32)

            sim1 = psum_sim.tile([P, CSH], f32, tag="sim1")
            sim2 = psum_sim.tile([P, CSH], f32, tag="sim2")

            # ---- DMA prologue: part-2 operands first, part-1 shards
            # interleaved behind them (the DMA path is serialized; order is
            # consumption order) ----
            chunk_tiles = {}
            s_ko = 0

            def emit_pred_chunk(t0, ncht):
                ch = im_pool.tile([P, KO, 4 * P], f32r, tag="imtile")
                nc.sync.dma_start(ch[:, :, :ncht * P],
                                  predT_r[:, :, t0 * P:(t0 + ncht) * P])
                for i in range(ncht):
                    chunk_tiles[t0 + i] = (ch, i)

            def emit_s_chunk():
                nonlocal s_ko
                if s_ko < KO:
                    nc.sync.dma_start(s_sb[:, s_ko:s_ko + 2, :],
                                      sT_r[:, s_ko:s_ko + 2, :])
                    s_ko += 2

            emit_pred_chunk(*PRED_CHUNKS[0])
            emit_pred_chunk(*PRED_CHUNKS[1])
            ap_all = const_pool.tile([P, NT2, P], f32)
            nc.sync.dma_start(ap_all[:], Apred_r)
            emit_pred_chunk(*PRED_CHUNKS[2])
            for ci in range(3, len(PRED_CHUNKS) - 1):
                emit_s_chunk()
                emit_pred_chunk(*PRED_CHUNKS[ci])
            emit_s_chunk()
            emit_s_chunk()
            emit_s_chunk()
            nc.sync.dma_start(a_all[:], Aim_r)
            emit_pred_chunk(*PRED_CHUNKS[-1])

            # ---- part 2: pred x s_pred ----
            pend = None  # (t, mx) of previous row tile
            for t in range(NT2):
                ch, i = chunk_tiles[t]
                mx = mx_pool.tile([P, CSH], f32, tag="mx")
                for ti in TILES2:
                    ps = psum_mm.tile([P, ti["cols"]], f32, tag="mm")
                    for ko in range(KO):
                        nc.tensor.matmul(
                            ps[:],
                            ch[:, ko, i * P:(i + 1) * P],
                            sp_sb[:, ko, ti["col0"]:ti["col0"] + ti["cols"]],
                            start=(ko == 0),
                            stop=(ko == KO - 1),
                        )
                    for (s0, n, wv, off) in ti["segs"]:
                        nc.vector.tensor_reduce(
                            out=mx[:, s0:s0 + n],
                            in_=ps[:, off:off + n * wv].rearrange(
                                "p (c w) -> p c w", w=wv),
                            axis=AX.X,
                            op=OP.max,
                        )
                if pend is not None:
                    nc.tensor.matmul(sim2[:], ap_all[:, pend[0], :], pend[1][:],
                                     start=(t == 1), stop=False)
                pend = (t, mx)
            nc.tensor.matmul(sim2[:], ap_all[:, pend[0], :], pend[1][:],
                             start=(NT2 == 1), stop=True)

            # ---- part 1: im x s ----
            pend = None
            chunk_tiles = {}
            for t0, ncht in IM_CHUNKS:
                ch = im_pool.tile([P, KO, 4 * P], f32r, tag="imtile")
                nc.sync.dma_start(ch[:, :, :ncht * P],
                                  imT_r[:, :, t0 * P:(t0 + ncht) * P])
                for i in range(ncht):
                    chunk_tiles[t0 + i] = (ch, i)
            for t in range(NT1):
                ch, i = chunk_tiles[t]
                mx = mx_pool.tile([P, CSH], f32, tag="mx")
                for ti in TILES1:
                    ps = psum_mm.tile([P, ti["cols"]], f32, tag="mm")
                    for ko in range(KO):
                        nc.tensor.matmul(
                            ps[:],
                            ch[:, ko, i * P:(i + 1) * P],
                            s_sb[:, ko, ti["col0"]:ti["col0"] + ti["cols"]],
                            start=(ko == 0),
                            stop=(ko == KO - 1),
                        )
                    for (s0, n, wv, off) in ti["segs"]:
                        nc.vector.tensor_reduce(
                            out=mx[:, s0:s0 + n],
                            in_=ps[:, off:off + n * wv].rearrange(
                                "p (c w) -> p c w", w=wv),
                            axis=AX.X,
                            op=OP.max,
                        )
                if pend is not None:
                    nc.tensor.matmul(sim1[:], a_all[:, pend[0], :], pend[1][:],
                                     start=(t == 1), stop=False)
                pend = (t, mx)
            nc.tensor.matmul(sim1[:], a_all[:, pend[0], :], pend[1][:],
                             start=(NT1 == 1), stop=True)

            # ---- tail: copy the two accumulators out (scaling on host) ----
            o = fin_pool.tile([P, 2 * CSH], f32)
            nc.scalar.copy(o[:, :CSH], sim1[:])
            nc.scalar.copy(o[:, CSH:], sim2[:])
            nc.sync.dma_start(out, o[:])

    nc.compile()
    return nc


def _get_nc(w1, w2):
    key = (w1, w2)
    if key not in _NC_CACHE:
        _NC_CACHE[key] = _build(w1, w2)
    return _NC_CACHE[key]


def _dup_pad_words(x, lens, width):
    # replace padded words with a copy of word 0 so that max over all words
    # == max over valid words (every row has >= 1 valid word)
    pad = np.arange(width)[None, :] >= lens[:, None]          # [B, W]
    return np.where(pad[:, :, None], x[:, :1, :], x)


def _sort_and_widths(lens):
    """Sort captions by length desc; bucket widths = max length per bucket."""
    order = np.argsort(-lens, kind="stable").astype(np.int64)     # [128]
    per_bucket = B // NBUCK                                       # 32 ranks
    bw = tuple(int(lens[order[per_bucket * k]]) for k in range(NBUCK))
    return order, bw


def _pack_core(x_fix, order, widths16, C, m):
    """[D, C] packed caption shard for core m (slot s holds order[8s+m])."""
    blocks = []
    for s in range(CSH):
        cap = order[NCORES * s + m]
        blocks.append(x_fix[cap, :widths16[s], :])                # [w, D]
    packed = np.concatenate(blocks, axis=0)                       # [C, D]
    return np.ascontiguousarray(packed.T)                         # [D, C]


LAST_RESULT = None


def kernel(im, im_l, s, s_l, pred, pred_l, s_pred, s_pred_l, _trace=False):
    from concourse.bass_utils import run_bass_kernel_spmd

    global LAST_RESULT
    im = np.asarray(im, np.float32)
    s = np.asarray(s, np.float32)
    pred = np.asarray(pred, np.float32)
    s_pred = np.asarray(s_pred, np.float32)
    im_l = np.asarray(im_l)
    s_l = np.asarray(s_l)
    pred_l = np.asarray(pred_l)
    s_pred_l = np.asarray(s_pred_l)

    s_fix = _dup_pad_words(s, s_l, W)
    sp_fix = _dup_pad_words(s_pred, s_pred_l, Wp)

    order1, w1 = _sort_and_widths(s_l)
    order2, w2 = _sort_and_widths(s_pred_l)
    widths16_1 = _slot_widths(w1)
    widths16_2 = _slot_widths(w2)
    _, C1 = _plan(widths16_1)
    _, C2 = _plan(widths16_2)

    imT = np.ascontiguousarray(im.reshape(BR, D).T)
    predT = np.ascontiguousarray(pred.reshape(BRp, D).T)

    Aim = np.zeros((BR, P), np.float32)
    Aim[np.arange(BR), np.arange(BR) // R] = 1.0
    Apred = np.zeros((BRp, P), np.float32)
    Apred[np.arange(BRp), np.arange(BRp) // Rp] = 1.0

    inv_im = (np.float32(1.0) / (im_l.astype(np.float32) + np.float32(EPS)))
    inv_pred = (np.float32(1.0) / (pred_l.astype(np.float32) + np.float32(EPS)))

    in_maps = []
    for m in range(NCORES):
        in_maps.append({
            "imT": imT,
            "predT": predT,
            "sT": _pack_core(s_fix, order1, widths16_1, C1, m),
            "spT": _pack_core(sp_fix, order2, widths16_2, C2, m),
            "Aim": Aim,
            "Apred": Apred,
        })

    nc = _get_nc(w1, w2)
    res = run_bass_kernel_spmd(
        nc, in_maps, core_ids=list(range(NCORES)), trace=_trace,
        trace_cores=list(range(NCORES)) if _trace else None,
    )
    LAST_RESULT = res

    scores = np.zeros((B, B), np.float32)
    slots = np.arange(CSH) * NCORES
    for m in range(NCORES):
        sims = res.results[m]["sims"]
        scores[:, order1[slots + m]] += (sims[:, :CSH] * inv_im[:, None]
                                         ).astype(np.float32)
        scores[:, order2[slots + m]] += (sims[:, CSH:] * inv_pred[:, None]
                                         ).astype(np.float32)

    diag = np.diagonal(scores).copy()[:, None]                 # [B, 1]
    cost_s = np.clip(MARGIN + scores - diag, 0.0, None)
    cost_im = np.clip(MARGIN + scores - diag.T, 0.0, None)
    np.fill_diagonal(cost_s, 0.0)
    np.fill_diagonal(cost_im, 0.0)
    loss = np.float32(cost_s.sum(dtype=np.float32) + cost_im.sum(dtype=np.float32))
    return loss, scores


# revision 12
# speedup vs baseline: 1.0975x; 1.0184x over previous
"""Trainium2 kernel for nn_ContrastiveLoss_matrix (cross-attention i2t contrastive loss).

Strategy (8 NeuronCores, SPMD, caption-axis sharding + ragged packing):
  - Captions are sorted by length (desc) and dealt round-robin into 16
    slots x 8 cores; slots are grouped into 4 equal-width buckets whose
    widths come from the actual length tensor at kernel-call time (the
    NEFF is compiled per width signature). This packs away ~40% of the
    padded-word matmul columns while every core runs the same kernel.
  - Einsum 'brd,cwd->bcrw' is a [BR, D] @ [D, cols] matmul in f32r
    (full-rate fp32 variant), D on the contraction (partition) axis.
  - Word masking: padded words are replaced host-side with a copy of
    word 0, so max over all packed words == max over valid words.
  - max over words: DVE segmented tensor_reduce per equal-width slot run.
  - sum over regions: PE matmul with a 0/1 indicator A[br, b] accumulated
    over row tiles, software-pipelined one iteration behind the main
    matmuls so the PE never waits on the DVE.
  - part 2 (pred) runs first (smallest startup transfer); part-1 operands
    stream in behind it with hand-interleaved DMA ordering.
  - host applies 1/(obj_num+eps), unpermutes the sorted caption columns,
    and computes the (tiny) margin loss in numpy.
"""

import numpy as np

B = 128
R = 36
W = 50
Rp = 25
Wp = 30
D = 1024
P = 128
KO = D // P          # 8 contraction chunks
NCORES = 8
CSH = B // NCORES    # 16 slots (captions) per core
NBUCK = 4            # slots per bucket = CSH // NBUCK
EPS = 1e-6
MARGIN = 0.2

BR = B * R           # 4608  -> 36 row tiles
BRp = B * Rp         # 3200  -> 25 row tiles
NT1 = BR // P        # 36
NT2 = BRp // P       # 25

_NC_CACHE = {}


def _chunks(n_tiles, first, step):
    """[(t0, ntiles), ...] covering range(n_tiles); first chunk small."""
    out = []
    t = 0
    size = first
    while t < n_tiles:
        sz = min(size, n_tiles - t)
        out.append((t, sz))
        t += sz
        size = step
    return out


def _slot_widths(bucket_w):
    return [bucket_w[s // (CSH // NBUCK)] for s in range(CSH)]


def _plan(widths):
    """Greedy-pack the 16 slots into PSUM tiles of <= 512 columns.

    Returns (tiles, total_cols); each tile is
    {'col0', 'cols', 'segs': [(slot0, nslots, width, rel_off), ...]}
    with segs being runs of equal-width slots."""
    tiles = []
    col = 0
    cur = None
    for s, wv in enumerate(widths):
        if cur is None or cur["cols"] + wv > 512:
            cur = {"col0": col, "cols": 0, "segs": []}
            tiles.append(cur)
        if cur["segs"] and cur["segs"][-1][2] == wv:
            s0, n, _, off = cur["segs"][-1]
            cur["segs"][-1] = (s0, n + 1, wv, off)
        else:
            cur["segs"].append((s, 1, wv, cur["cols"]))
        cur["cols"] += wv
        col += wv
    return tiles, col


def _build(w1, w2):
    import concourse.tile as tile
    from concourse import bacc, mybir

    f32 = mybir.dt.float32
    f32r = mybir.dt.float32r
    AX = mybir.AxisListType
    OP = mybir.AluOpType

    TILES1, COLS1 = _plan(_slot_widths(w1))
    TILES2, COLS2 = _plan(_slot_widths(w2))

    nc = bacc.Bacc("TRN2", target_bir_lowering=False, debug=False)

    imT = nc.dram_tensor("imT", [D, BR], f32r, kind="ExternalInput").ap()
    predT = nc.dram_tensor("predT", [D, BRp], f32r, kind="ExternalInput").ap()
    sT = nc.dram_tensor("sT", [D, COLS1], f32r, kind="ExternalInput").ap()
    spT = nc.dram_tensor("spT", [D, COLS2], f32r, kind="ExternalInput").ap()
    Aim = nc.dram_tensor("Aim", [BR, P], f32, kind="ExternalInput").ap()
    Apred = nc.dram_tensor("Apred", [BRp, P], f32, kind="ExternalInput").ap()
    out = nc.dram_tensor("sims", [P, 2 * CSH], f32, kind="ExternalOutput").ap()

    imT_r = imT.rearrange("(ko p) n -> p ko n", p=P)      # [128, 8, 4608]
    predT_r = predT.rearrange("(ko p) n -> p ko n", p=P)  # [128, 8, 3200]
    sT_r = sT.rearrange("(ko p) n -> p ko n", p=P)        # [128, 8, C1]
    spT_r = spT.rearrange("(ko p) n -> p ko n", p=P)      # [128, 8, C2]
    Aim_r = Aim.rearrange("(t p) m -> p t m", p=P)        # [128, 36, 128]
    Apred_r = Apred.rearrange("(t p) m -> p t m", p=P)    # [128, 25, 128]

    IM_CHUNKS = _chunks(NT1, 2, 4)    # small first chunks for fast start
    PRED_CHUNKS = _chunks(NT2, 1, 2)

    with tile.TileContext(nc) as tc:
        with (
            tc.tile_pool(name="const", bufs=1) as const_pool,
            tc.tile_pool(name="imt", bufs=6) as im_pool,
            tc.tile_pool(name="mx", bufs=4) as mx_pool,
            tc.tile_pool(name="fin", bufs=1) as fin_pool,
            tc.tile_pool(name="mm", bufs=6, space="PSUM") as psum_mm,
            tc.tile_pool(name="sim", bufs=1, space="PSUM") as psum_sim,
        ):
            sp_sb = const_pool.tile([P, KO, COLS2], f32r)
            for ko in range(0, KO, 2):   # small chunks -> compute starts early
                nc.sync.dma_start(sp_sb[:, ko:ko + 2, :], spT_r[:, ko:ko + 2, :])
            s_sb = const_pool.tile([P, KO, COLS1], f32r)
            a_all = const_pool.tile([P, NT1, P], f32)

            sim1 = psum_sim.tile([P, CSH], f32, tag="sim1")
            sim2 = psum_sim.tile([P, CSH], f32, tag="sim2")

            # ---- DMA prologue: part-2 operands first, part-1 shards
            # interleaved behind them (the DMA path is serialized; order is
            # consumption order) ----
            chunk_tiles = {}
            s_ko = 0

            def emit_pred_chunk(t0, ncht):
                ch = im_pool.tile([P, KO, 4 * P], f32r, tag="imtile")
                nc.sync.dma_start(ch[:, :, :ncht * P],
                                  predT_r[:, :, t0 * P:(t0 + ncht) * P])
                for i in range(ncht):
                    chunk_tiles[t0 + i] = (ch, i)

            def emit_s_chunk():
                nonlocal s_ko
                if s_ko < KO:
                    nc.sync.dma_start(s_sb[:, s_ko:s_ko + 2, :],
                                      sT_r[:, s_ko:s_ko + 2, :])
                    s_ko += 2

            emit_pred_chunk(*PRED_CHUNKS[0])
            emit_pred_chunk(*PRED_CHUNKS[1])
            ap_all = const_pool.tile([P, NT2, P], f32)
            nc.sync.dma_start(ap_all[:], Apred_r)
            emit_pred_chunk(*PRED_CHUNKS[2])
            # spread part-1's s-shard chunks through the pred stream (one s
            # chunk per two pred chunks keeps pred supply ~matched to its
            # consumption rate on the serialized DMA path)
            im_iter = iter(IM_CHUNKS)
            im_emitted = []

            def emit_im_chunk():
                try:
                    t0, ncht = next(im_iter)
                except StopIteration:
                    return
                ch = im_pool.tile([P, KO, 4 * P], f32r, tag="imtile")
                nc.sync.dma_start(ch[:, :, :ncht * P],
                                  imT_r[:, :, t0 * P:(t0 + ncht) * P])
                for i in range(ncht):
                    im_emitted.append(None)
                    chunk_tiles_im[t0 + i] = (ch, i)

            chunk_tiles_im = {}
            for ci in range(3, len(PRED_CHUNKS)):
                if ci % 2 == 1:
                    emit_s_chunk()
                emit_pred_chunk(*PRED_CHUNKS[ci])
            emit_s_chunk()
            emit_s_chunk()
            emit_s_chunk()
            emit_im_chunk()
            emit_im_chunk()
            nc.sync.dma_start(a_all[:], Aim_r)

            # ---- part 2: pred x s_pred ----
            pend = None  # (t, mx) of previous row tile
            for t in range(NT2):
                ch, i = chunk_tiles[t]
                mx = mx_pool.tile([P, CSH], f32, tag="mx")
                for ti in TILES2:
                    ps = psum_mm.tile([P, ti["cols"]], f32, tag="mm")
                    for ko in range(KO):
                        nc.tensor.matmul(
                            ps[:],
                            ch[:, ko, i * P:(i + 1) * P],
                            sp_sb[:, ko, ti["col0"]:ti["col0"] + ti["cols"]],
                            start=(ko == 0),
                            stop=(ko == KO - 1),
                        )
                    for (s0, n, wv, off) in ti["segs"]:
                        nc.vector.tensor_reduce(
                            out=mx[:, s0:s0 + n],
                            in_=ps[:, off:off + n * wv].rearrange(
                                "p (c w) -> p c w", w=wv),
                            axis=AX.X,
                            op=OP.max,
                        )
                if pend is not None:
                    nc.tensor.matmul(sim2[:], ap_all[:, pend[0], :], pend[1][:],
                                     start=(t == 1), stop=False)
                pend = (t, mx)
            nc.tensor.matmul(sim2[:], ap_all[:, pend[0], :], pend[1][:],
                             start=(NT2 == 1), stop=True)

            # ---- part 1: im x s ----
            pend = None
            while True:
                before = len(chunk_tiles_im)
                emit_im_chunk()
                if len(chunk_tiles_im) == before:
                    break
            for t in range(NT1):
                ch, i = chunk_tiles_im[t]
                mx = mx_pool.tile([P, CSH], f32, tag="mx")
                for ti in TILES1:
                    ps = psum_mm.tile([P, ti["cols"]], f32, tag="mm")
                    for ko in range(KO):
                        nc.tensor.matmul(
                            ps[:],
                            ch[:, ko, i * P:(i + 1) * P],
                            s_sb[:, ko, ti["col0"]:ti["col0"] + ti["cols"]],
                            start=(ko == 0),
                            stop=(ko == KO - 1),
                        )
                    for (s0, n, wv, off) in ti["segs"]:
                        nc.vector.tensor_reduce(
                            out=mx[:, s0:s0 + n],
                            in_=ps[:, off:off + n * wv].rearrange(
                                "p (c w) -> p c w", w=wv),
                            axis=AX.X,
                            op=OP.max,
                        )
                if pend is not None:
                    nc.tensor.matmul(sim1[:], a_all[:, pend[0], :], pend[1][:],
                                     start=(t == 1), stop=False)
                pend = (t, mx)
            nc.tensor.matmul(sim1[:], a_all[:, pend[0], :], pend[1][:],
                             start=(NT1 == 1), stop=True)

            # ---- tail: copy the two accumulators out (scaling on host) ----
            o = fin_pool.tile([P, 2 * CSH], f32)
            nc.scalar.copy(o[:, :CSH], sim1[:])
            nc.scalar.copy(o[:, CSH:], sim2[:])
            nc.sync.dma_start(out, o[:])

    nc.compile()
    return nc


def _get_nc(w1, w2):
    key = (w1, w2)
    if key not in _NC_CACHE:
        _NC_CACHE[key] = _build(w1, w2)
    return _NC_CACHE[key]


def _dup_pad_words(x, lens, width):
    # replace padded words with a copy of word 0 so that max over all words
    # == max over valid words (every row has >= 1 valid word)
    pad = np.arange(width)[None, :] >= lens[:, None]          # [B, W]
    return np.where(pad[:, :, None], x[:, :1, :], x)


def _sort_and_widths(lens):
    """Sort captions by length desc; bucket widths = max length per bucket."""
    order = np.argsort(-lens, kind="stable").astype(np.int64)     # [128]
    per_bucket = B // NBUCK                                       # 32 ranks
    bw = tuple(int(lens[order[per_bucket * k]]) for k in range(NBUCK))
    return order, bw


def _pack_core(x_fix, order, widths16, C, m):
    """[D, C] packed caption shard for core m (slot s holds order[8s+m])."""
    blocks = []
    for s in range(CSH):
        cap = order[NCORES * s + m]
        blocks.append(x_fix[cap, :widths16[s], :])                # [w, D]
    packed = np.concatenate(blocks, axis=0)                       # [C, D]
    return np.ascontiguousarray(packed.T)                         # [D, C]


LAST_RESULT = None


def kernel(im, im_l, s, s_l, pred, pred_l, s_pred, s_pred_l, _trace=False):
    from concourse.bass_utils import run_bass_kernel_spmd

    global LAST_RESULT
    im = np.asarray(im, np.float32)
    s = np.asarray(s, np.float32)
    pred = np.asarray(pred, np.float32)
    s_pred = np.asarray(s_pred, np.float32)
    im_l = np.asarray(im_l)
    s_l = np.asarray(s_l)
    pred_l = np.asarray(pred_l)
    s_pred_l = np.asarray(s_pred_l)

    s_fix = _dup_pad_words(s, s_l, W)
    sp_fix = _dup_pad_words(s_pred, s_pred_l, Wp)

    order1, w1 = _sort_and_widths(s_l)
    order2, w2 = _sort_and_widths(s_pred_l)
    widths16_1 = _slot_widths(w1)
    widths16_2 = _slot_widths(w2)
    _, C1 = _plan(widths16_1)
    _, C2 = _plan(widths16_2)

    imT = np.ascontiguousarray(im.reshape(BR, D).T)
    predT = np.ascontiguousarray(pred.reshape(BRp, D).T)

    Aim = np.zeros((BR, P), np.float32)
    Aim[np.arange(BR), np.arange(BR) // R] = 1.0
    Apred = np.zeros((BRp, P), np.float32)
    Apred[np.arange(BRp), np.arange(BRp) // Rp] = 1.0

    inv_im = (np.float32(1.0) / (im_l.astype(np.float32) + np.float32(EPS)))
    inv_pred = (np.float32(1.0) / (pred_l.astype(np.float32) + np.float32(EPS)))

    in_maps = []
    for m in range(NCORES):
        in_maps.append({
            "imT": imT,
            "predT": predT,
            "sT": _pack_core(s_fix, order1, widths16_1, C1, m),
            "spT": _pack_core(sp_fix, order2, widths16_2, C2, m),
            "Aim": Aim,
            "Apred": Apred,
        })

    nc = _get_nc(w1, w2)
    res = run_bass_kernel_spmd(
        nc, in_maps, core_ids=list(range(NCORES)), trace=_trace,
        trace_cores=list(range(NCORES)) if _trace else None,
    )
    LAST_RESULT = res

    scores = np.zeros((B, B), np.float32)
    slots = np.arange(CSH) * NCORES
    for m in range(NCORES):
        sims = res.results[m]["sims"]
        scores[:, order1[slots + m]] += (sims[:, :CSH] * inv_im[:, None]
                                         ).astype(np.float32)
        scores[:, order2[slots + m]] += (sims[:, CSH:] * inv_pred[:, None]
                                         ).astype(np.float32)

    diag = np.diagonal(scores).copy()[:, None]                 # [B, 1]
    cost_s = np.clip(MARGIN + scores - diag, 0.0, None)
    cost_im = np.clip(MARGIN + scores - diag.T, 0.0, None)
    np.fill_diagonal(cost_s, 0.0)
    np.fill_diagonal(cost_im, 0.0)
    loss = np.float32(cost_s.sum(dtype=np.float32) + cost_im.sum(dtype=np.float32))
    return loss, scores
